# revision 3
# baseline (speedup 1.0000x reference)
"""Trainium2 kernel for nn_BernNet_47364899340878.

Math note (why the device kernel is just the MLP):
  The reference computes  out = sum_{j=0..K} c_j * relu(temp_j) * L^j (2I-L)^{K-j} h
  with c_j = C(K,j)/2^K and h = relu(x@W1+b1)@W2+b2.  The graded inputs pin
  temp = ones (spec fill "ones"), so relu(temp_j) = 1 for all j.  L and
  (2I - L) are commuting polynomials in the normalized adjacency, so the
  binomial theorem gives

      sum_j C(K,j) L^j (2I-L)^{K-j} = (L + 2I - L)^K = (2I)^K = 2^K I,

  i.e. the whole K=10 Bernstein propagation is exactly the identity map and
  out == h.  A non-ones temp (never the case for the graded inputs) falls
  back to a host implementation of the propagation for correctness.

Device kernel: h = relu(x@W1+b1)@W2+b2 and log_softmax(h), row-sharded over
8 NeuronCores (12500 rows each).

Perf design (v2, from baseline trace analysis at 121.5us):
  The baseline was bound by (a) fp32 input traffic (25.6MB/core vs the
  ~358 GB/s HBM-per-core limit), (b) 25 small input DMAs with 2KB
  descriptors costing ~800ns issue each on the SP sequencer, and (c) the
  PE running at its cold 1.2GHz clock for the whole kernel (per-block
  matmul bursts of ~2.5us never hit the ~3.4us HAM activity window, so
  the clock gate never opened to 2.4GHz).

  v2 changes (correctness headroom: harness gate is rel_err < 2e-2,
  fp16 lands at ~5e-4):
  - whole pipeline in fp16: input 12.5MB/core, output 2MB/core,
    ~41us DMA roofline vs 83us for fp32.
  - 6 big input DMAs per core (4x2500 + 1500 + 1000 rows), each fully
    contiguous in DRAM with 20KB-per-partition descriptors (near
    line-rate), issued on the SP HWDGE queue; weights+outputs ride the
    ACT HWDGE queue.
  - contraction uses 125 partitions x 4 chunks = 500 exactly (no 512 pad).
  - outputs (raw+logp, fp16) batched per input block: one DMA per block
    with 3200B-per-partition segments.
  - a ~80-matmul pre-warm burst on W1 bridges the PE from the first
    weight arrival to the first input block, so HAM un-throttles the PE
    clock before real work starts and it never re-throttles (steady-state
    per-block idle gaps are well under the ~3.4us MID window).
  - bias+relu fused on DVE writes h^T directly as fp16 (the 65th W1
    column/ones-row trick folds b2 into the second matmul, as baseline);
    log_softmax skips max-subtraction (logits bounded, exp can't
    overflow); Exp/Ln pinned to one ACT table set (one table load).
"""

import numpy as np

_N = 100000
_FIN = 500
_HID = 64
_CLS = 40
_NCORES = 8
_RPC = _N // _NCORES  # 12500 rows per core
_KP = 125  # contraction partitions per chunk
_KC = 4  # contraction chunks (125*4 = 500)
_SUB = 500  # rows per compute sub-block (PSUM-bank sized)
_SI = 125  # rows per mm2 subtile
_NSI = 4
_BLOCKS = (2500, 2500, 2500, 2500, 1500, 1000)  # input-DMA block sizes
_PREWARM = 80  # PE warm-up matmuls

_CACHE = {}


def _build_bass():
    from contextlib import ExitStack

    import concourse.bacc as bacc
    import concourse.mybir as mybir
    import concourse.tile as tile

    fp32 = mybir.dt.float32
    fp16 = mybir.dt.float16
    AF = mybir.ActivationFunctionType
    OP = mybir.AluOpType

    # Table-set pinning: ACT function tables are loaded as named sets and a
    # set switch costs ~1.3-2.7us.  Exp and Ln both live in the
    # "natural_log_exp_and_others" set, but the default insertion pass picks
    # each function's first containing set, so an Exp/Ln mix reloads on every
    # switch.  Restricting Exp/Ln to their shared set (keeping every set's
    # positional id intact) makes the whole kernel need exactly one load.
    class _PinnedActBacc(bacc.Bacc):
        def insert_act_table_loads(self):
            import bass_rust as _bass_rust
            from concourse.hw_specs import get_activation_tables

            has_activation = any(
                isinstance(i, mybir.InstActivation)
                for b in self.main_func.blocks
                for i in b.instructions
            )
            if not has_activation:
                return
            shared = {AF.Exp, AF.Ln}
            tables = []
            for name, fns in get_activation_tables(self.m.arch).items():
                if name != "natural_log_exp_and_others":
                    fns = fns - shared
                tables.append((name, fns))
            _bass_rust.insert_act_table_loads(self, tables)

    nc = _PinnedActBacc()
    xts = [
        nc.dram_tensor(f"xt{b}", [_KP, _KC, r], fp16, kind="ExternalInput")
        for b, r in enumerate(_BLOCKS)
    ]
    w1 = nc.dram_tensor("w1", [_KP, _KC, _HID + 1], fp16, kind="ExternalInput")
    b1 = nc.dram_tensor("b1", [_HID + 1, 1], fp32, kind="ExternalInput")
    w2 = nc.dram_tensor("w2", [_HID + 1, _CLS], fp16, kind="ExternalInput")
    # per-block output [p, sub, j, si, c]: each partition's raw+logp data is
    # one contiguous (nsub*320B) run in DRAM -> large-segment output DMAs.
    # Host permutes (sub, si, p) -> rows afterwards.
    bths = [
        nc.dram_tensor(
            f"both{b}", [_SI, r // _SUB, 2, _NSI, _CLS], fp16, kind="ExternalOutput"
        )
        for b, r in enumerate(_BLOCKS)
    ]

    with tile.TileContext(nc) as tc, ExitStack() as ctx:
        const = ctx.enter_context(tc.tile_pool(name="const", bufs=1))
        xpool = ctx.enter_context(tc.tile_pool(name="xin", bufs=3))
        hpool = ctx.enter_context(tc.tile_pool(name="hrelu", bufs=3))
        epool = ctx.enter_context(tc.tile_pool(name="expv", bufs=3))
        opool = ctx.enter_context(tc.tile_pool(name="outs", bufs=2))
        spool = ctx.enter_context(tc.tile_pool(name="sums", bufs=3))
        lpool = ctx.enter_context(tc.tile_pool(name="lsub", bufs=3))
        pp1 = ctx.enter_context(tc.tile_pool(name="ps1", bufs=3, space="PSUM"))
        pp2 = ctx.enter_context(tc.tile_pool(name="ps2", bufs=3, space="PSUM"))
        ppw = ctx.enter_context(tc.tile_pool(name="psw", bufs=1, space="PSUM"))

        # first (big) input DMA on the SP queue ASAP; weights ride the ACT
        # queue so they land while block 0 streams.
        x_sb = []
        x_sb.append(xpool.tile([_KP, _KC, _BLOCKS[0]], fp16, tag="xt", name="xt_sb0"))
        nc.sync.dma_start(x_sb[0][:], xts[0][:])

        w1_sb = const.tile([_KP, _KC, _HID + 1], fp16)
        nc.scalar.dma_start(w1_sb[:], w1[:])
        b1_sb = const.tile([_HID + 1, 1], fp32)
        nc.scalar.dma_start(b1_sb[:], b1[:])
        w2_sb = const.tile([_HID + 1, _CLS], fp16)
        nc.scalar.dma_start(w2_sb[:], w2[:])

        # PE pre-warm: a burst of dummy matmuls on W1 bridges the gap from
        # weight arrival (~7.5us) to block-0 arrival (~16us).  HAM opens the
        # clock gate after ~3.4us of sustained PE activity, so the real
        # matmuls start at 2.4GHz.  Result is never read.
        pw_ps = ppw.tile([_HID + 1, _HID + 1], fp32)
        for i in range(_PREWARM):
            nc.tensor.matmul(
                pw_ps[:],
                w1_sb[:, 0, :],
                w1_sb[:, i % _KC, :],
                start=(i == 0),
                stop=(i == _PREWARM - 1),
            )

        # remaining input DMAs queue up behind block 0 on the SP queue
        # (xpool bufs=3 lets 2 of them start before block 0 is consumed).
        for b in range(1, len(_BLOCKS)):
            x_sb.append(xpool.tile([_KP, _KC, _BLOCKS[b]], fp16, tag="xt", name=f"xt_sb{b}"))
            nc.sync.dma_start(x_sb[b][:], xts[b][:])

        for b, rows in enumerate(_BLOCKS):
            nsub = rows // _SUB
            cmb = opool.tile([_SI, nsub, 2, _NSI, _CLS], fp16, tag="cmb")
            for s in range(nsub):
                # h^T = (W1p^T @ x^T) : [65, 500], accumulated over 4 chunks
                ht_ps = pp1.tile([_HID + 1, _SUB], fp32)
                for kc in range(_KC):
                    nc.tensor.matmul(
                        ht_ps[:],
                        w1_sb[:, kc, :],
                        x_sb[b][:, kc, s * _SUB : (s + 1) * _SUB],
                        start=(kc == 0),
                        stop=(kc == _KC - 1),
                    )
                # fused bias+relu on DVE: max(ht + b1, 0); row 64 = 1 folds
                # b2 into mm2 via the W2 b2-row.
                ht_relu = hpool.tile([_HID + 1, _SUB], fp16)
                nc.vector.tensor_scalar(
                    out=ht_relu[:], in0=ht_ps[:], scalar1=b1_sb[:], scalar2=0.0,
                    op0=OP.add, op1=OP.max,
                )
                # out = h_relu_aug^T.T @ W2_aug : 4 subtiles of 125 rows
                o_ps = pp2.tile([_SI, _NSI, _CLS], fp32)
                for si in range(_NSI):
                    nc.tensor.matmul(
                        o_ps[:, si, :],
                        ht_relu[:, si * _SI : (si + 1) * _SI],
                        w2_sb[:],
                    )
                # raw logits (fp16) into the combined per-block output tile
                nc.vector.tensor_copy(cmb[:, s, 0], o_ps[:])
                # log_softmax without max-subtraction (logits bounded)
                e_sb = epool.tile([_SI, _NSI, _CLS], fp32)
                nc.scalar.activation(e_sb[:], o_ps[:], AF.Exp)
                ssum = spool.tile([_SI, _NSI], fp32)
                nc.vector.tensor_reduce(
                    out=ssum[:], in_=e_sb[:], op=OP.add, axis=mybir.AxisListType.X,
                )
                lse = lpool.tile([_SI, _NSI], fp32)
                nc.scalar.activation(lse[:], ssum[:], AF.Ln)
                nc.vector.tensor_sub(
                    cmb[:, s, 1],
                    o_ps[:],
                    lse[:, :, None].broadcast_to([_SI, _NSI, _CLS]),
                )
            # one output DMA per input block from the ACT HWDGE queue
            nc.scalar.dma_start(bths[b][:], cmb[:])

    nc.finalize()
    return nc


def _get_bass():
    if "nc" not in _CACHE:
        _CACHE["nc"] = _build_bass()
    return _CACHE["nc"]


def _host_prep(x, W1, b1, W2, b2):
    """Build per-core fp16 input shards + augmented fp16 weights."""
    x16 = np.asarray(x, np.float32).astype(np.float16)
    w1a = np.zeros((_FIN, _HID + 1), np.float16)
    w1a[:, :_HID] = np.asarray(W1, np.float32).astype(np.float16)
    w1p = np.ascontiguousarray(
        w1a.reshape(_KC, _KP, _HID + 1).transpose(1, 0, 2)
    )  # [125, 4, 65]
    b1a = np.zeros((_HID + 1, 1), np.float32)
    b1a[:_HID, 0] = np.asarray(b1, np.float32)
    b1a[_HID, 0] = 1.0
    w2a = np.zeros((_HID + 1, _CLS), np.float16)
    w2a[:_HID] = np.asarray(W2, np.float32).astype(np.float16)
    w2a[_HID] = np.asarray(b2, np.float32).astype(np.float16)

    in_maps = []
    for c in range(_NCORES):
        m = {"w1": w1p, "b1": b1a, "w2": w2a}
        start = c * _RPC
        for b, r in enumerate(_BLOCKS):
            seg = x16[start : start + r]  # [r, 500]
            # [r, kc, p] -> [p, kc, r]
            m[f"xt{b}"] = np.ascontiguousarray(
                seg.reshape(r, _KC, _KP).transpose(2, 1, 0)
            )
            start += r
        in_maps.append(m)
    return in_maps


def _unshard(results):
    """results: list of per-core dicts with both{b} [125, nsub, 2, 4, 40]."""
    out = np.empty((_N, _CLS), np.float32)
    lp = np.empty((_N, _CLS), np.float32)
    for c in range(_NCORES):
        start = c * _RPC
        for b, r in enumerate(_BLOCKS):
            a = np.asarray(results[c][f"both{b}"], np.float32)
            # [p, sub, j, si, c] -> [sub, si, p, j, c] -> rows
            a = a.transpose(1, 3, 0, 2, 4).reshape(r, 2, _CLS)
            out[start : start + r] = a[:, 0]
            lp[start : start + r] = a[:, 1]
            start += r
    return lp, out


def _bern_prop_host(h, edge_index, theta):
    """Fallback: full Bernstein propagation on host (only if temp != ones)."""
    from math import comb

    n = h.shape[0]
    src = np.asarray(edge_index[0], np.int64)
    dst = np.asarray(edge_index[1], np.int64)
    deg = np.bincount(src, minlength=n).astype(np.float32)
    dis = np.where(deg > 0, 1.0 / np.sqrt(np.maximum(deg, 1.0)), 0.0).astype(
        np.float32
    )

    def anorm(v):
        msg = v[src] * dis[src][:, None]
        out = np.zeros_like(v)
        np.add.at(out, dst, msg)
        return out * dis[:, None]

    K = len(theta) - 1
    tmp = [h]
    for _ in range(K):
        t = tmp[-1]
        tmp.append(t + anorm(t))
    c = np.array([comb(K, j) / 2.0**K for j in range(K + 1)], np.float32)
    acc = np.zeros_like(h)
    for j in range(K, 0, -1):
        s = acc + c[j] * theta[j] * tmp[K - j]
        acc = s - anorm(s)
    return c[0] * theta[0] * tmp[K] + acc


def kernel(x, edge_index, W1, b1, W2, b2, temp):
    from concourse.bass_utils import run_bass_kernel_spmd

    in_maps = _host_prep(x, W1, b1, W2, b2)
    nc = _get_bass()
    res = run_bass_kernel_spmd(nc, in_maps, core_ids=list(range(_NCORES)))
    lp, out = _unshard(res.results)

    theta = np.maximum(np.asarray(temp, np.float32), 0.0)
    if not np.allclose(theta, 1.0):
        # General-temp path: device computed h; propagate on host, then
        # recompute log_softmax.
        out = _bern_prop_host(out.astype(np.float32), edge_index, theta)
        m = out.max(axis=1, keepdims=True)
        lp = out - (np.log(np.exp(out - m).sum(axis=1, keepdims=True)) + m)
        lp = lp.astype(np.float32)

    return lp, out


# revision 4
# speedup vs baseline: 1.8104x; 1.8104x over previous
"""Trainium2 kernel for nn_BernNet_47364899340878.

Math note (why the device kernel is just the MLP):
  The reference computes  out = sum_{j=0..K} c_j * relu(temp_j) * L^j (2I-L)^{K-j} h
  with c_j = C(K,j)/2^K and h = relu(x@W1+b1)@W2+b2.  The graded inputs pin
  temp = ones (spec fill "ones"), so relu(temp_j) = 1 for all j.  L and
  (2I - L) are commuting polynomials in the normalized adjacency, so the
  binomial theorem gives

      sum_j C(K,j) L^j (2I-L)^{K-j} = (L + 2I - L)^K = (2I)^K = 2^K I,

  i.e. the whole K=10 Bernstein propagation is exactly the identity map and
  out == h.  A non-ones temp (never the case for the graded inputs) falls
  back to a host implementation of the propagation for correctness.

Device kernel: h = relu(x@W1+b1)@W2+b2 and log_softmax(h), row-sharded over
8 NeuronCores (12500 rows each).

Perf design (v2, from baseline trace analysis at 121.5us):
  The baseline was bound by (a) fp32 input traffic (25.6MB/core vs the
  ~358 GB/s HBM-per-core limit), (b) 25 small input DMAs with 2KB
  descriptors costing ~800ns issue each on the SP sequencer, and (c) the
  PE running at its cold 1.2GHz clock for the whole kernel (per-block
  matmul bursts of ~2.5us never hit the ~3.4us HAM activity window, so
  the clock gate never opened to 2.4GHz).

  v2 changes (correctness headroom: harness gate is rel_err < 2e-2,
  fp16 lands at ~5e-4):
  - whole pipeline in fp16: input 12.5MB/core, output 2MB/core,
    ~41us DMA roofline vs 83us for fp32.
  - 6 big input DMAs per core (4x2500 + 1500 + 1000 rows), each fully
    contiguous in DRAM with 20KB-per-partition descriptors (near
    line-rate), issued on the SP HWDGE queue; weights+outputs ride the
    ACT HWDGE queue.
  - contraction uses 125 partitions x 4 chunks = 500 exactly (no 512 pad).
  - outputs (raw+logp, fp16) batched per input block: one DMA per block
    with 3200B-per-partition segments.
  - a ~80-matmul pre-warm burst on W1 bridges the PE from the first
    weight arrival to the first input block, so HAM un-throttles the PE
    clock before real work starts and it never re-throttles (steady-state
    per-block idle gaps are well under the ~3.4us MID window).
  - bias+relu fused on DVE writes h^T directly as fp16 (the 65th W1
    column/ones-row trick folds b2 into the second matmul, as baseline);
    log_softmax skips max-subtraction (logits bounded, exp can't
    overflow); Exp/Ln pinned to one ACT table set (one table load).
"""

import numpy as np

_N = 100000
_FIN = 500
_HID = 64
_CLS = 40
_NCORES = 8
_RPC = _N // _NCORES  # 12500 rows per core
_KP = 128  # contraction partitions per chunk (128 -> full 16-way DMA spray)
_KPAD = 512  # padded contraction (500 -> 512)
_KC = 4  # contraction chunks (128*4 = 512)
_SUB = 500  # rows per compute sub-block (PSUM-bank sized)
_SI = 125  # rows per mm2 subtile
_NSI = 4
_OP = 128  # output-tile partition dim (125 used, padded for 16-way spray)
_BLOCKS = (1000, 2500, 2500, 2500, 2500, 1500)  # input-DMA block sizes
_PREWARM = 60  # PE warm-up matmuls

_CACHE = {}


def _build_bass():
    from contextlib import ExitStack

    import concourse.bacc as bacc
    import concourse.mybir as mybir
    import concourse.tile as tile

    fp32 = mybir.dt.float32
    fp16 = mybir.dt.float16
    AF = mybir.ActivationFunctionType
    OP = mybir.AluOpType

    # Table-set pinning: ACT function tables are loaded as named sets and a
    # set switch costs ~1.3-2.7us.  Exp and Ln both live in the
    # "natural_log_exp_and_others" set, but the default insertion pass picks
    # each function's first containing set, so an Exp/Ln mix reloads on every
    # switch.  Restricting Exp/Ln to their shared set (keeping every set's
    # positional id intact) makes the whole kernel need exactly one load.
    class _PinnedActBacc(bacc.Bacc):
        def insert_act_table_loads(self):
            import bass_rust as _bass_rust
            from concourse.hw_specs import get_activation_tables

            has_activation = any(
                isinstance(i, mybir.InstActivation)
                for b in self.main_func.blocks
                for i in b.instructions
            )
            if not has_activation:
                return
            shared = {AF.Exp, AF.Ln}
            tables = []
            for name, fns in get_activation_tables(self.m.arch).items():
                if name != "natural_log_exp_and_others":
                    fns = fns - shared
                tables.append((name, fns))
            _bass_rust.insert_act_table_loads(self, tables)

    nc = _PinnedActBacc()
    xts = [
        nc.dram_tensor(f"xt{b}", [_KP, _KC, r], fp16, kind="ExternalInput")
        for b, r in enumerate(_BLOCKS)
    ]
    w1 = nc.dram_tensor("w1", [_KP, _KC, _HID + 1], fp16, kind="ExternalInput")
    b1 = nc.dram_tensor("b1", [_HID + 1, 1], fp32, kind="ExternalInput")
    w2 = nc.dram_tensor("w2", [_HID + 1, _CLS], fp16, kind="ExternalInput")
    # per-block output [p, sub, j, si, c]: each partition's raw+logp data is
    # one contiguous (nsub*320B) run in DRAM -> large-segment output DMAs.
    # Host permutes (sub, si, p) -> rows afterwards.
    bths = [
        nc.dram_tensor(
            f"both{b}", [_OP, r // _SUB, 2, _NSI, _CLS], fp16, kind="ExternalOutput"
        )
        for b, r in enumerate(_BLOCKS)
    ]

    with tile.TileContext(nc) as tc, ExitStack() as ctx:
        const = ctx.enter_context(tc.tile_pool(name="const", bufs=1))
        xpool = ctx.enter_context(tc.tile_pool(name="xin", bufs=3))
        hpool = ctx.enter_context(tc.tile_pool(name="hrelu", bufs=3))
        epool = ctx.enter_context(tc.tile_pool(name="expv", bufs=3))
        opool = ctx.enter_context(tc.tile_pool(name="outs", bufs=2))
        spool = ctx.enter_context(tc.tile_pool(name="sums", bufs=3))
        lpool = ctx.enter_context(tc.tile_pool(name="lsub", bufs=3))
        pp1 = ctx.enter_context(tc.tile_pool(name="ps1", bufs=3, space="PSUM"))
        pp2 = ctx.enter_context(tc.tile_pool(name="ps2", bufs=3, space="PSUM"))
        ppw = ctx.enter_context(tc.tile_pool(name="psw", bufs=1, space="PSUM"))

        # first (big) input DMA on the SP queue ASAP; weights ride the ACT
        # queue so they land while block 0 streams.
        x_sb = []
        x_sb.append(xpool.tile([_KP, _KC, _BLOCKS[0]], fp16, tag="xt", name="xt_sb0"))
        nc.sync.dma_start(x_sb[0][:], xts[0][:])

        w1_sb = const.tile([_KP, _KC, _HID + 1], fp16)
        nc.scalar.dma_start(w1_sb[:], w1[:])
        b1_sb = const.tile([_HID + 1, 1], fp32)
        nc.scalar.dma_start(b1_sb[:], b1[:])
        w2_sb = const.tile([_HID + 1, _CLS], fp16)
        nc.scalar.dma_start(w2_sb[:], w2[:])

        # PE pre-warm: a burst of dummy matmuls on W1 bridges the gap from
        # weight arrival (~7.5us) to block-0 arrival (~16us).  HAM opens the
        # clock gate after ~3.4us of sustained PE activity, so the real
        # matmuls start at 2.4GHz.  Result is never read.
        pw_ps = ppw.tile([_HID + 1, _HID + 1], fp32)
        for i in range(_PREWARM):
            nc.tensor.matmul(
                pw_ps[:],
                w1_sb[:, 0, :],
                w1_sb[:, i % _KC, :],
                start=(i == 0),
                stop=(i == _PREWARM - 1),
            )

        # remaining input DMAs queue up behind block 0 on the SP queue
        # (xpool bufs=3 lets 2 of them start before block 0 is consumed).
        for b in range(1, len(_BLOCKS)):
            x_sb.append(xpool.tile([_KP, _KC, _BLOCKS[b]], fp16, tag="xt", name=f"xt_sb{b}"))
            nc.sync.dma_start(x_sb[b][:], xts[b][:])

        for b, rows in enumerate(_BLOCKS):
            nsub = rows // _SUB
            cmb = opool.tile([_OP, nsub, 2, _NSI, _CLS], fp16, tag="cmb")
            for s in range(nsub):
                # h^T = (W1p^T @ x^T) : [65, 500], accumulated over 4 chunks
                ht_ps = pp1.tile([_HID + 1, _SUB], fp32)
                for kc in range(_KC):
                    nc.tensor.matmul(
                        ht_ps[:],
                        w1_sb[:, kc, :],
                        x_sb[b][:, kc, s * _SUB : (s + 1) * _SUB],
                        start=(kc == 0),
                        stop=(kc == _KC - 1),
                    )
                # fused bias+relu on DVE: max(ht + b1, 0); row 64 = 1 folds
                # b2 into mm2 via the W2 b2-row.
                ht_relu = hpool.tile([_HID + 1, _SUB], fp16)
                nc.vector.tensor_scalar(
                    out=ht_relu[:], in0=ht_ps[:], scalar1=b1_sb[:], scalar2=0.0,
                    op0=OP.add, op1=OP.max,
                )
                # out = h_relu_aug^T.T @ W2_aug : 4 subtiles of 125 rows
                o_ps = pp2.tile([_SI, _NSI, _CLS], fp32)
                for si in range(_NSI):
                    nc.tensor.matmul(
                        o_ps[:, si, :],
                        ht_relu[:, si * _SI : (si + 1) * _SI],
                        w2_sb[:],
                    )
                # raw logits (fp16) into the combined per-block output tile
                nc.vector.tensor_copy(cmb[:_SI, s, 0], o_ps[:])
                # log_softmax without max-subtraction (logits bounded)
                e_sb = epool.tile([_SI, _NSI, _CLS], fp32)
                nc.scalar.activation(e_sb[:], o_ps[:], AF.Exp)
                ssum = spool.tile([_SI, _NSI], fp32)
                nc.vector.tensor_reduce(
                    out=ssum[:], in_=e_sb[:], op=OP.add, axis=mybir.AxisListType.X,
                )
                lse = lpool.tile([_SI, _NSI], fp32)
                nc.scalar.activation(lse[:], ssum[:], AF.Ln)
                nc.vector.tensor_sub(
                    cmb[:_SI, s, 1],
                    o_ps[:],
                    lse[:, :, None].broadcast_to([_SI, _NSI, _CLS]),
                )
            # one output DMA per input block from the ACT HWDGE queue
            nc.scalar.dma_start(bths[b][:], cmb[:])

    nc.finalize()
    return nc


def _get_bass():
    if "nc" not in _CACHE:
        _CACHE["nc"] = _build_bass()
    return _CACHE["nc"]


def _host_prep(x, W1, b1, W2, b2):
    """Build per-core fp16 input shards + augmented fp16 weights."""
    x16 = np.zeros((_N, _KPAD), np.float16)
    x16[:, :_FIN] = np.asarray(x, np.float32).astype(np.float16)
    w1a = np.zeros((_KPAD, _HID + 1), np.float16)
    w1a[:_FIN, :_HID] = np.asarray(W1, np.float32).astype(np.float16)
    w1p = np.ascontiguousarray(
        w1a.reshape(_KC, _KP, _HID + 1).transpose(1, 0, 2)
    )  # [128, 4, 65]
    b1a = np.zeros((_HID + 1, 1), np.float32)
    b1a[:_HID, 0] = np.asarray(b1, np.float32)
    b1a[_HID, 0] = 1.0
    w2a = np.zeros((_HID + 1, _CLS), np.float16)
    w2a[:_HID] = np.asarray(W2, np.float32).astype(np.float16)
    w2a[_HID] = np.asarray(b2, np.float32).astype(np.float16)

    in_maps = []
    for c in range(_NCORES):
        m = {"w1": w1p, "b1": b1a, "w2": w2a}
        start = c * _RPC
        for b, r in enumerate(_BLOCKS):
            seg = x16[start : start + r]  # [r, 512]
            # [r, kc, p] -> [p, kc, r]
            m[f"xt{b}"] = np.ascontiguousarray(
                seg.reshape(r, _KC, _KP).transpose(2, 1, 0)
            )
            start += r
        in_maps.append(m)
    return in_maps


def _unshard(results):
    """results: list of per-core dicts with both{b} [125, nsub, 2, 4, 40]."""
    out = np.empty((_N, _CLS), np.float32)
    lp = np.empty((_N, _CLS), np.float32)
    for c in range(_NCORES):
        start = c * _RPC
        for b, r in enumerate(_BLOCKS):
            a = np.asarray(results[c][f"both{b}"][:_SI], np.float32)
            # [p, sub, j, si, c] -> [sub, si, p, j, c] -> rows
            a = a.transpose(1, 3, 0, 2, 4).reshape(r, 2, _CLS)
            out[start : start + r] = a[:, 0]
            lp[start : start + r] = a[:, 1]
            start += r
    return lp, out


def _bern_prop_host(h, edge_index, theta):
    """Fallback: full Bernstein propagation on host (only if temp != ones)."""
    from math import comb

    n = h.shape[0]
    src = np.asarray(edge_index[0], np.int64)
    dst = np.asarray(edge_index[1], np.int64)
    deg = np.bincount(src, minlength=n).astype(np.float32)
    dis = np.where(deg > 0, 1.0 / np.sqrt(np.maximum(deg, 1.0)), 0.0).astype(
        np.float32
    )

    def anorm(v):
        msg = v[src] * dis[src][:, None]
        out = np.zeros_like(v)
        np.add.at(out, dst, msg)
        return out * dis[:, None]

    K = len(theta) - 1
    tmp = [h]
    for _ in range(K):
        t = tmp[-1]
        tmp.append(t + anorm(t))
    c = np.array([comb(K, j) / 2.0**K for j in range(K + 1)], np.float32)
    acc = np.zeros_like(h)
    for j in range(K, 0, -1):
        s = acc + c[j] * theta[j] * tmp[K - j]
        acc = s - anorm(s)
    return c[0] * theta[0] * tmp[K] + acc


def kernel(x, edge_index, W1, b1, W2, b2, temp):
    from concourse.bass_utils import run_bass_kernel_spmd

    in_maps = _host_prep(x, W1, b1, W2, b2)
    nc = _get_bass()
    res = run_bass_kernel_spmd(nc, in_maps, core_ids=list(range(_NCORES)))
    lp, out = _unshard(res.results)

    theta = np.maximum(np.asarray(temp, np.float32), 0.0)
    if not np.allclose(theta, 1.0):
        # General-temp path: device computed h; propagate on host, then
        # recompute log_softmax.
        out = _bern_prop_host(out.astype(np.float32), edge_index, theta)
        m = out.max(axis=1, keepdims=True)
        lp = out - (np.log(np.exp(out - m).sum(axis=1, keepdims=True)) + m)
        lp = lp.astype(np.float32)

    return lp, out
